# revision 4
# baseline (speedup 1.0000x reference)
"""Trainium2 Bass kernel: nn_DifferentiableSelector (soft top-K w/ refractory damping).

Data-parallel over batch: 512 rows -> 64 rows/core on 8 NeuronCores. Each row of
T=32768 is split into two 16384 halves so all 128 SBUF partitions are used
(partition p<64 holds row p cols [0,16384); partition p>=64 holds row p-64 cols
[16384,32768)).

Math: y0 = sigmoid(scores/temp); budget_r = clip(sum_i y0[r,i], 1e-6);
y = y0 * min(K/budget, 1); then R=4 damping iters y *= min(2/(1+y+roll(y,-d)),1);
y[:,0] = 0.

Damping-identity property (load-bearing): if budget_r >= 2K = 128 for every row,
then min(K/budget,1) <= 0.5 (correctly-rounded fp32 div of K by >=2K is <= 0.5),
so every y <= 0.5 (rounded product of factors <= 0.5 and <= 1.0 is <= 0.5), so
s = fl(y[i]+y[i+d]) <= 1, fl(1+s) <= 2, fl(2/(1+s)) >= 1, and
min(2/(1+s), 1.0) == 1.0 *exactly*; y*1.0 is bitwise identity. Inductively the
entire damping loop is an exact fp32 no-op. For N(0,1)-like scores,
budget ~ T/2 = 16384, margin ~128x. The device exports per-row budgets; the host
checks budget >= 256 on all rows and otherwise falls back to a full numpy
evaluation of the reference semantics (exact for arbitrary inputs).
"""

import numpy as np

B, T = 512, 32768
K = 64.0
R_REFRACTORY = 4
N_CORES = 8
ROWS = B // N_CORES  # 64 rows per core
H = T // 2  # 16384 half-row length
P = 128
W = 2048  # column tile width
NT = H // W

_NC_CACHE: dict = {}


def _build_nc(inv_temp: float, reps: int = 1):
    """Build + compile the per-core Bass program (SPMD, same NEFF on all cores).

    reps > 1 emits the body multiple times back-to-back for benchmarking
    (per-rep steady-state time = total / reps, amortizing dispatch overhead).
    """
    from contextlib import ExitStack

    import concourse.bacc as bacc
    import concourse.tile as tile
    from concourse import mybir

    f32 = mybir.dt.float32
    nc = bacc.Bacc(
        "TRN2",
        target_bir_lowering=False,
        debug=False,
        enable_asserts=False,
        num_devices=N_CORES,
    )

    scores_h = nc.dram_tensor("scores", [ROWS, T], f32, kind="ExternalInput")
    wsum_h = nc.dram_tensor("wsum", [P, P], f32, kind="ExternalInput")
    y_h = nc.dram_tensor("y", [ROWS, T], f32, kind="ExternalOutput")
    bud_h = nc.dram_tensor("budgets", [ROWS, 1], f32, kind="ExternalOutput")

    # [h, r, c] iterates (h, r) outer-to-inner, matching SBUF partition
    # p = h*64 + r: partition p<64 <- row p first half, p>=64 <- row p-64
    # second half. dma_start only requires equal total sizes.
    s_view = scores_h.rearrange("r (h c) -> h r c", h=2)
    y_view = y_h.rearrange("r (h c) -> h r c", h=2)

    with tile.TileContext(nc) as tc, ExitStack() as ctx:
        res = ctx.enter_context(tc.tile_pool(name="res", bufs=1))
        stats = ctx.enter_context(tc.tile_pool(name="stats", bufs=2))
        consts = ctx.enter_context(tc.tile_pool(name="consts", bufs=1))
        psum = ctx.enter_context(tc.tile_pool(name="psum", bufs=2, space="PSUM"))

        wsum_t = consts.tile([P, P], f32)
        nc.sync.dma_start(wsum_t[:], wsum_h[:, :])

        for _rep in range(reps):
            y0 = res.tile([P, H], f32)
            partials = stats.tile([P, NT], f32)
            # Phase 1: stream scores in, sigmoid in place, fused row-partials.
            for i in range(NT):
                sl = slice(i * W, (i + 1) * W)
                nc.sync.dma_start(y0[:, sl], s_view[:, :, sl])
                nc.scalar.activation(
                    y0[:, sl],
                    y0[:, sl],
                    mybir.ActivationFunctionType.Sigmoid,
                    scale=float(inv_temp),
                    accum_out=partials[:, i : i + 1],
                )

            # Row budgets: sum partials, then one PE matmul both pair-sums the
            # two half-rows (partitions p and p+64) and broadcasts the result
            # to all 128 partitions: out[p] = total[p%64] + total[p%64+64].
            total = stats.tile([P, 1], f32)
            nc.vector.tensor_reduce(
                total[:], partials[:], axis=mybir.AxisListType.X, op=mybir.AluOpType.add
            )
            bud_ps = psum.tile([P, 1], f32)
            nc.tensor.matmul(bud_ps[:], wsum_t[:], total[:], start=True, stop=True)
            bud = stats.tile([P, 1], f32)
            nc.vector.tensor_scalar_max(bud[:], bud_ps[:], 1e-6)  # clip(budget,1e-6)
            rb = stats.tile([P, 1], f32)
            nc.vector.reciprocal(rb[:], bud[:])
            g = stats.tile([P, 1], f32)
            nc.vector.tensor_scalar(  # g = min(K/budget, 1)
                g[:],
                rb[:],
                K,
                1.0,
                op0=mybir.AluOpType.mult,
                op1=mybir.AluOpType.min,
            )
            nc.sync.dma_start(bud_h[:, :], bud[0:ROWS, 0:1])

            # Phase 2: scale in place per tile, zero col 0 of each row, stream out.
            for i in range(NT):
                sl = slice(i * W, (i + 1) * W)
                nc.vector.tensor_scalar_mul(y0[:, sl], y0[:, sl], g[:, 0:1])
                if i == 0:
                    nc.vector.memset(y0[0:ROWS, 0:1], 0.0)
                nc.sync.dma_start(y_view[:, :, sl], y0[:, sl])

    nc.compile()
    return nc


def _get_nc(inv_temp: float, reps: int = 1):
    key = (round(float(inv_temp), 9), reps)
    if key not in _NC_CACHE:
        _NC_CACHE[key] = _build_nc(inv_temp, reps)
    return _NC_CACHE[key]


def _wsum_matrix() -> np.ndarray:
    # lhsT[k, m] = 1 iff k % 64 == m % 64  ->  out[m] = total[m%64] + total[m%64+64]
    return np.tile(np.eye(ROWS, dtype=np.float32), (2, 2))


def _temp_from_log(log_temperature) -> np.float32:
    lt = np.float32(np.asarray(log_temperature, dtype=np.float32).reshape(()))
    return np.float32(np.clip(np.exp(lt, dtype=np.float32), 0.1, 10.0))


def _reference_fallback(scores: np.ndarray, temp: np.float32) -> np.ndarray:
    # Exact general-case evaluation (mirrors reference.py in fp32 numpy).
    y = 1.0 / (1.0 + np.exp(-(scores / temp), dtype=np.float32))
    y = y.astype(np.float32)
    budget = np.clip(np.sum(y, axis=1, keepdims=True, dtype=np.float32), 1e-6, None)
    y = y * np.minimum(np.float32(K) / budget, np.float32(1.0))
    t = scores.shape[1]
    for d in range(1, min(R_REFRACTORY + 1, t)):
        shift = np.roll(y, -d, axis=1)
        y = y * np.minimum(2.0 / (1.0 + y + shift), 1.0).astype(np.float32)
    y = y.astype(np.float32)
    y[:, 0] = 0.0
    return y


def kernel(scores: np.ndarray, log_temperature: np.ndarray) -> np.ndarray:
    from concourse.bass_utils import run_bass_kernel_spmd

    scores = np.ascontiguousarray(scores, dtype=np.float32)
    assert scores.shape == (B, T), scores.shape
    temp = _temp_from_log(log_temperature)
    inv_temp = np.float32(1.0) / temp

    nc = _get_nc(float(inv_temp))
    wsum = _wsum_matrix()
    in_maps = [
        {"scores": scores[c * ROWS : (c + 1) * ROWS], "wsum": wsum}
        for c in range(N_CORES)
    ]
    res = run_bass_kernel_spmd(nc, in_maps, list(range(N_CORES))).results
    y = np.concatenate([res[c]["y"] for c in range(N_CORES)], axis=0)
    budgets = np.concatenate([res[c]["budgets"][:, 0] for c in range(N_CORES)])

    # Damping is an exact fp32 identity iff every row budget >= 2K (see module
    # docstring); 256 adds 2x margin over the required 128. If violated (never,
    # for randn-scale inputs), recompute everything faithfully on the host.
    if not np.all(budgets >= 256.0):
        return _reference_fallback(scores, temp)
    return y


# revision 6
# speedup vs baseline: 6.3006x; 6.3006x over previous
"""Trainium2 Bass kernel: nn_DifferentiableSelector (soft top-K w/ refractory damping).

Data-parallel over batch: 512 rows -> 64 rows/core on 8 NeuronCores.

Device layout: each core's [64, 32768] block is viewed flat-contiguously as
[128, 16384] (partition p = 2r + h holds half h of row r), so both the input
and output DMA are single fully-contiguous 8MB transfers — by far the fastest
DMA shape (partition-interleaved layouts measured 6-30x slower). Sigmoid runs
as 8 column-tiled out-of-place ACT passes with fused row-partial accumulation
(accum_out); one PE matmul against a 0/1 pair matrix both sums each row's two
half-partitions and broadcasts the budget to both; the scale g = K/budget is
applied as one full-width (even-length, 2x-mode) DVE tensor_scalar pass, then
column 0 is overwritten via a masked per-partition factor to zero each row's
first element.

Math: y0 = sigmoid(scores/temp); budget_r = clip(sum_i y0[r,i], 1e-6);
y = y0 * min(K/budget, 1); then R=4 damping iters
y *= min(2/(1+y+roll(y,-d)), 1); y[:,0] = 0.

Damping-identity property (load-bearing): if budget_r >= 2K = 128 for every
row, then min(K/budget,1) <= 0.5 (correctly-rounded fp32 div), so every
y <= 0.5, so s = fl(y[i]+y[i+d]) <= 1, fl(1+s) <= 2, fl(2/(1+s)) >= 1, and
min(2/(1+s), 1.0) == 1.0 *exactly*; y*1.0 is bitwise identity. Inductively the
whole damping loop is an exact fp32 no-op. For N(0,1)-like scores,
budget ~ T/2 = 16384 (margin ~128x over the threshold). The device exports the
raw per-row sums; the host checks sum >= 256 for every row and otherwise falls
back to a full numpy evaluation of the reference semantics (exact for
arbitrary inputs; never taken for the spec'd input distribution). The same
check makes clip(budget, 1e-6) and min(K/budget, 1) identities on the device
path, so the device computes g = K * reciprocal(sum) directly.
"""

import numpy as np

B, T = 512, 32768
K = 64.0
R_REFRACTORY = 4
N_CORES = 8
ROWS = B // N_CORES  # 64 rows per core
H = T // 2
P = 128

ACT_TILES = 8
DVE_TILES = 2

_NC_CACHE: dict = {}


def _build_nc(
    inv_temp: float,
    reps: int = 1,
    act_tiles: int = ACT_TILES,
    dve_tiles: int = DVE_TILES,
    warm_table: bool = True,
):
    from contextlib import ExitStack

    import concourse.bacc as bacc
    import concourse.tile as tile
    from concourse import mybir

    f32 = mybir.dt.float32
    nc = bacc.Bacc(
        "TRN2",
        target_bir_lowering=False,
        debug=False,
        enable_asserts=False,
        num_devices=N_CORES,
    )
    scores_h = nc.dram_tensor("scores", [ROWS, T], f32, kind="ExternalInput")
    wsum_h = nc.dram_tensor("wsum", [P, P], f32, kind="ExternalInput")
    mask_h = nc.dram_tensor("mask", [P, 1], f32, kind="ExternalInput")
    y_h = nc.dram_tensor("y", [ROWS, T], f32, kind="ExternalOutput")
    bud_h = nc.dram_tensor("budgets", [P, 1], f32, kind="ExternalOutput")

    # [128, H], fully contiguous in DRAM: partition p = 2r+h
    s_c = scores_h.rearrange("r (h c) -> (r h) c", h=2)
    y_c = y_h.rearrange("r (h c) -> (r h) c", h=2)

    with tile.TileContext(nc) as tc, ExitStack() as ctx:
        res = ctx.enter_context(tc.tile_pool(name="res", bufs=1))
        stats = ctx.enter_context(tc.tile_pool(name="stats", bufs=2))
        consts = ctx.enter_context(tc.tile_pool(name="consts", bufs=1))
        psum = ctx.enter_context(tc.tile_pool(name="psum", bufs=2, space="PSUM"))

        wsum_t = consts.tile([P, P], f32)
        nc.sync.dma_start(wsum_t[:], wsum_h[:, :])
        mask_t = consts.tile([P, 1], f32)
        nc.sync.dma_start(mask_t[:], mask_h[:, :])
        if warm_table:
            # Load the sigmoid ACT table set while the first big DMA streams.
            wtile = consts.tile([P, 1], f32)
            nc.vector.memset(wtile[:], 0.0)
            nc.scalar.activation(
                wtile[:], wtile[:], mybir.ActivationFunctionType.Sigmoid
            )

        for _rep in range(reps):
            y0 = res.tile([P, H], f32)
            y1 = res.tile([P, H], f32)

            nc.sync.dma_start(y0[:, :], s_c[:, :])

            partials = stats.tile([P, act_tiles], f32, tag="partials")
            wt = H // act_tiles
            for i in range(act_tiles):
                sl = slice(i * wt, (i + 1) * wt)
                nc.scalar.activation(
                    y1[:, sl],
                    y0[:, sl],
                    mybir.ActivationFunctionType.Sigmoid,
                    scale=float(inv_temp),
                    accum_out=partials[:, i : i + 1],
                )
            total = stats.tile([P, 1], f32, tag="total")
            nc.vector.tensor_reduce(
                total[:], partials[:], axis=mybir.AxisListType.X, op=mybir.AluOpType.add
            )

            # pair-sum + broadcast: bud[p] = total[p] + total[p^1] (same row)
            bud_ps = psum.tile([P, 1], f32, tag="budps")
            nc.tensor.matmul(bud_ps[:], wsum_t[:], total[:, 0:1], start=True, stop=True)
            bud = stats.tile([P, 1], f32, tag="bud")
            nc.vector.tensor_copy(bud[:], bud_ps[:])  # raw row sums (exported)
            rb = stats.tile([P, 1], f32, tag="rb")
            nc.vector.reciprocal(rb[:], bud[:])
            g = stats.tile([P, 1], f32, tag="g")
            nc.vector.tensor_scalar_mul(g[:], rb[:], K)  # g = K/budget
            gm = stats.tile([P, 1], f32, tag="gm")  # g with row-start zeroing
            nc.vector.tensor_scalar(
                gm[:],
                rb[:],
                mask_t[:, 0:1],
                K,
                op0=mybir.AluOpType.mult,
                op1=mybir.AluOpType.mult,
            )

            # full-width even-length TS (2x mode), then overwrite col 0
            dt = H // dve_tiles
            for i in range(dve_tiles):
                sl = slice(i * dt, (i + 1) * dt)
                nc.vector.tensor_scalar_mul(y0[:, sl], y1[:, sl], g[:, 0:1])
            nc.vector.tensor_mul(y0[:, 0:1], y1[:, 0:1], gm[:, 0:1])

            nc.sync.dma_start(y_c[:, :], y0[:, :])
            nc.gpsimd.dma_start(bud_h[:, :], bud[:, 0:1])
    nc.compile()
    return nc


def _get_nc(inv_temp: float, reps: int = 1, **kw):
    key = (round(float(inv_temp), 9), reps, tuple(sorted(kw.items())))
    if key not in _NC_CACHE:
        _NC_CACHE[key] = _build_nc(inv_temp, reps, **kw)
    return _NC_CACHE[key]


def _wsum_matrix() -> np.ndarray:
    # wsum[k, m] = 1 iff k//2 == m//2: sums partition pairs (2r, 2r+1) and
    # broadcasts back to both — one matmul does the whole budget reduction.
    return np.kron(np.eye(ROWS, dtype=np.float32), np.ones((2, 2), np.float32))


def _mask_matrix() -> np.ndarray:
    # 0 at even partitions (they hold each row's column 0), else 1
    m = np.ones((P, 1), np.float32)
    m[0::2, 0] = 0.0
    return m


def _temp_from_log(log_temperature) -> np.float32:
    lt = np.float32(np.asarray(log_temperature, dtype=np.float32).reshape(()))
    return np.float32(np.clip(np.exp(lt, dtype=np.float32), 0.1, 10.0))


def _reference_fallback(scores: np.ndarray, temp: np.float32) -> np.ndarray:
    # Exact general-case evaluation (mirrors reference.py in fp32 numpy).
    y = 1.0 / (1.0 + np.exp(-(scores / temp), dtype=np.float32))
    y = y.astype(np.float32)
    budget = np.clip(np.sum(y, axis=1, keepdims=True, dtype=np.float32), 1e-6, None)
    y = y * np.minimum(np.float32(K) / budget, np.float32(1.0))
    t = scores.shape[1]
    for d in range(1, min(R_REFRACTORY + 1, t)):
        shift = np.roll(y, -d, axis=1)
        y = y * np.minimum(2.0 / (1.0 + y + shift), 1.0).astype(np.float32)
    y = y.astype(np.float32)
    y[:, 0] = 0.0
    return y


def kernel(scores: np.ndarray, log_temperature: np.ndarray) -> np.ndarray:
    from concourse.bass_utils import run_bass_kernel_spmd

    scores = np.ascontiguousarray(scores, dtype=np.float32)
    assert scores.shape == (B, T), scores.shape
    temp = _temp_from_log(log_temperature)
    inv_temp = np.float32(1.0) / temp

    nc = _get_nc(float(inv_temp))
    wsum = _wsum_matrix()
    mask = _mask_matrix()
    in_maps = [
        {"scores": scores[c * ROWS : (c + 1) * ROWS], "wsum": wsum, "mask": mask}
        for c in range(N_CORES)
    ]
    res = run_bass_kernel_spmd(nc, in_maps, list(range(N_CORES))).results
    y = np.concatenate([res[c]["y"] for c in range(N_CORES)], axis=0)
    # budgets[2r] = raw sum of row r (per core)
    budgets = np.concatenate(
        [res[c]["budgets"][0::2, 0] for c in range(N_CORES)]
    )

    # Damping is an exact fp32 identity iff every row budget >= 2K (see module
    # docstring); 256 adds 2x margin over the required 128. If violated (never,
    # for randn-scale inputs), recompute everything faithfully on the host.
    if not np.all(budgets >= 256.0):
        return _reference_fallback(scores, temp)
    return y


# revision 7
# speedup vs baseline: 7.2550x; 1.1515x over previous
"""Trainium2 Bass kernel: nn_DifferentiableSelector (soft top-K w/ refractory damping).

Data-parallel over batch: 512 rows -> 64 rows/core on 8 NeuronCores.

Device layout ("two contiguous row-chunks"): each core's [64, 32768] block is
split into 2 contiguous address-range chunks of 32 rows. Chunk k, viewed as
[128, 4096], holds rows 32k..32k+31 with row 32k+j on partitions
[4j, 4j+4) — so every DMA is one fully-contiguous 4MB transfer (measured
6-30x faster on this target than partition-interleaved patterns), and chunk
k+1's input DMA overlaps chunk k's compute while chunk k's output DMA overlaps
chunk k+1's compute. Per chunk: sigmoid as 2048-wide out-of-place ACT tiles
with fused row-partial accumulation (accum_out), one PE matmul against a 0/1
block matrix to group-sum + broadcast the row budgets, reciprocal straight
from PSUM, then one full-width (even-length, 2x-mode) DVE tensor_scalar scale
pass; column 0 of each row is then overwritten via a masked per-partition
factor to implement y[:, 0] = 0.

Math: y0 = sigmoid(scores/temp); budget_r = clip(sum_i y0[r,i], 1e-6);
y = y0 * min(K/budget, 1); then R=4 damping iters
y *= min(2/(1+y+roll(y,-d)), 1); y[:,0] = 0.

Damping-identity property (load-bearing): if budget_r >= 2K = 128 for every
row, then min(K/budget,1) <= 0.5 (correctly-rounded fp32 div), so every
y <= 0.5, so s = fl(y[i]+y[i+d]) <= 1, fl(1+s) <= 2, fl(2/(1+s)) >= 1, and
min(2/(1+s), 1.0) == 1.0 *exactly*; y*1.0 is bitwise identity. Inductively the
whole damping loop is an exact fp32 no-op. For N(0,1)-like scores,
budget ~ T/2 = 16384 (margin ~128x over the threshold). The device exports the
raw per-row sums; the host checks sum >= 256 for every row and otherwise falls
back to a full numpy evaluation of the reference semantics (exact for
arbitrary inputs; never taken for the spec'd input distribution). The same
check makes clip(budget, 1e-6) and min(K/budget, 1) identities on the device
path, so the device computes g = K * reciprocal(sum) directly.
"""

import numpy as np

B, T = 512, 32768
K = 64.0
R_REFRACTORY = 4
N_CORES = 8
ROWS = B // N_CORES  # 64 rows per core
P = 128

NCHUNK = 2
RPC = ROWS // NCHUNK  # 32 rows per chunk
GS = P // RPC  # 4 partitions per row within a chunk
WC = RPC * T // P  # 8192 free width per chunk
ACT_W = 2048  # ACT tile width

_NC_CACHE: dict = {}


def _build_nc(inv_temp: float, reps: int = 1):
    from contextlib import ExitStack

    import concourse.bacc as bacc
    import concourse.tile as tile
    from concourse import mybir

    f32 = mybir.dt.float32
    nc = bacc.Bacc(
        "TRN2",
        target_bir_lowering=False,
        debug=False,
        enable_asserts=False,
        num_devices=N_CORES,
    )
    scores_h = nc.dram_tensor("scores", [ROWS, T], f32, kind="ExternalInput")
    wsum_h = nc.dram_tensor("wsum", [P, P], f32, kind="ExternalInput")
    mask_h = nc.dram_tensor("mask", [P, 1], f32, kind="ExternalInput")
    y_h = nc.dram_tensor("y", [ROWS, T], f32, kind="ExternalOutput")
    bud_h = nc.dram_tensor("budgets", [NCHUNK, P], f32, kind="ExternalOutput")

    # [nchunk, 128, Wc] flat-contiguous chunk views
    s_k = scores_h.rearrange("r (q w) -> (r q) w", w=WC).rearrange(
        "(k p) w -> k p w", p=P
    )
    y_k = y_h.rearrange("r (q w) -> (r q) w", w=WC).rearrange("(k p) w -> k p w", p=P)

    with tile.TileContext(nc) as tc, ExitStack() as ctx:
        inp = ctx.enter_context(tc.tile_pool(name="inp", bufs=2))
        sig = ctx.enter_context(tc.tile_pool(name="sig", bufs=2))
        outp = ctx.enter_context(tc.tile_pool(name="outp", bufs=2))
        stats = ctx.enter_context(tc.tile_pool(name="stats", bufs=4))
        consts = ctx.enter_context(tc.tile_pool(name="consts", bufs=1))
        psum = ctx.enter_context(tc.tile_pool(name="psum", bufs=4, space="PSUM"))

        wsum_t = consts.tile([P, P], f32)
        nc.sync.dma_start(wsum_t[:], wsum_h[:, :])
        mask_t = consts.tile([P, 1], f32)
        nc.sync.dma_start(mask_t[:], mask_h[:, :])
        # Load the sigmoid ACT table set while the first big DMA streams.
        wtile = consts.tile([P, 1], f32)
        nc.vector.memset(wtile[:], 0.0)
        nc.scalar.activation(wtile[:], wtile[:], mybir.ActivationFunctionType.Sigmoid)

        for _rep in range(reps):
            for k in range(NCHUNK):
                t_in = inp.tile([P, WC], f32, tag="in")
                nc.sync.dma_start(t_in[:], s_k[k, :, :])
                t_sig = sig.tile([P, WC], f32, tag="sig")
                ntile = WC // ACT_W
                partials = stats.tile([P, ntile], f32, tag="partials")
                for i in range(ntile):
                    sl = slice(i * ACT_W, (i + 1) * ACT_W)
                    nc.scalar.activation(
                        t_sig[:, sl],
                        t_in[:, sl],
                        mybir.ActivationFunctionType.Sigmoid,
                        scale=float(inv_temp),
                        accum_out=partials[:, i : i + 1],
                    )
                total = stats.tile([P, 1], f32, tag="total")
                nc.vector.tensor_reduce(
                    total[:],
                    partials[:],
                    axis=mybir.AxisListType.X,
                    op=mybir.AluOpType.add,
                )
                # group-sum + broadcast: bud[p] = sum of total over p's 4-group
                bud_ps = psum.tile([P, 1], f32, tag="budps")
                nc.tensor.matmul(
                    bud_ps[:], wsum_t[:], total[:, 0:1], start=True, stop=True
                )
                rb = stats.tile([P, 1], f32, tag="rb")
                nc.vector.reciprocal(rb[:], bud_ps[:])
                gm = stats.tile([P, 1], f32, tag="gm")  # K/b with row-start zeroing
                nc.vector.tensor_scalar(
                    gm[:],
                    rb[:],
                    mask_t[:, 0:1],
                    K,
                    op0=mybir.AluOpType.mult,
                    op1=mybir.AluOpType.mult,
                )
                t_out = outp.tile([P, WC], f32, tag="out")
                # plain single-op TS with precomputed g keeps 2x mode
                g = stats.tile([P, 1], f32, tag="g")
                nc.vector.tensor_scalar_mul(g[:], rb[:], K)
                nc.vector.tensor_scalar_mul(t_out[:, :], t_sig[:, :], g[:, 0:1])
                nc.vector.tensor_mul(t_out[:, 0:1], t_sig[:, 0:1], gm[:, 0:1])
                nc.sync.dma_start(y_k[k, :, :], t_out[:])
                # export raw row sums (off critical path)
                bud = stats.tile([P, 1], f32, tag="bud")
                nc.vector.tensor_copy(bud[:], bud_ps[:])
                nc.gpsimd.dma_start(bud_h[k : k + 1, :], bud[:, 0:1])
    nc.compile()
    return nc


def _get_nc(inv_temp: float, reps: int = 1):
    key = (round(float(inv_temp), 9), reps)
    if key not in _NC_CACHE:
        _NC_CACHE[key] = _build_nc(inv_temp, reps)
    return _NC_CACHE[key]


def _wsum_matrix() -> np.ndarray:
    # wsum[k, m] = 1 iff k//GS == m//GS: sums each row's GS partitions and
    # broadcasts back to all of them — one matmul does the whole reduction.
    return np.kron(np.eye(P // GS, dtype=np.float32), np.ones((GS, GS), np.float32))


def _mask_matrix() -> np.ndarray:
    # 0 at partitions holding a row start (p % GS == 0), else 1
    m = np.ones((P, 1), np.float32)
    m[0::GS, 0] = 0.0
    return m


def _temp_from_log(log_temperature) -> np.float32:
    lt = np.float32(np.asarray(log_temperature, dtype=np.float32).reshape(()))
    return np.float32(np.clip(np.exp(lt, dtype=np.float32), 0.1, 10.0))


def _reference_fallback(scores: np.ndarray, temp: np.float32) -> np.ndarray:
    # Exact general-case evaluation (mirrors reference.py in fp32 numpy).
    y = 1.0 / (1.0 + np.exp(-(scores / temp), dtype=np.float32))
    y = y.astype(np.float32)
    budget = np.clip(np.sum(y, axis=1, keepdims=True, dtype=np.float32), 1e-6, None)
    y = y * np.minimum(np.float32(K) / budget, np.float32(1.0))
    t = scores.shape[1]
    for d in range(1, min(R_REFRACTORY + 1, t)):
        shift = np.roll(y, -d, axis=1)
        y = y * np.minimum(2.0 / (1.0 + y + shift), 1.0).astype(np.float32)
    y = y.astype(np.float32)
    y[:, 0] = 0.0
    return y


def kernel(scores: np.ndarray, log_temperature: np.ndarray) -> np.ndarray:
    from concourse.bass_utils import run_bass_kernel_spmd

    scores = np.ascontiguousarray(scores, dtype=np.float32)
    assert scores.shape == (B, T), scores.shape
    temp = _temp_from_log(log_temperature)
    inv_temp = np.float32(1.0) / temp

    nc = _get_nc(float(inv_temp))
    wsum = _wsum_matrix()
    mask = _mask_matrix()
    in_maps = [
        {"scores": scores[c * ROWS : (c + 1) * ROWS], "wsum": wsum, "mask": mask}
        for c in range(N_CORES)
    ]
    res = run_bass_kernel_spmd(nc, in_maps, list(range(N_CORES))).results
    y = np.concatenate([res[c]["y"] for c in range(N_CORES)], axis=0)
    # budgets[k, GS*j] = raw sum of row RPC*k + j (per core)
    budgets = np.concatenate(
        [res[c]["budgets"][:, 0::GS].reshape(-1) for c in range(N_CORES)]
    )

    # Damping is an exact fp32 identity iff every row budget >= 2K (see module
    # docstring); 256 adds 2x margin over the required 128. If violated (never,
    # for randn-scale inputs), recompute everything faithfully on the host.
    if not np.all(budgets >= 256.0):
        return _reference_fallback(scores, temp)
    return y
